# revision 7
# baseline (speedup 1.0000x reference)
"""Trainium2 Bass kernel for an embedding + multi-kernel-size conv1d +
attention-pooling encoder (ENCAML).

Math (per batch b):
  xc = embed[x[b]].T                               # [E=100, L=2500]
  for each branch i (conv kernel size k in (3,5,7,9)):
    h  = conv1d_same(xc, cw_i) + cb_i              # [F=256, L]
    x1 = tanh(h)
    logits = uw_i @ x1                             # [C=50, L]
    alpha  = softmax(logits, axis=L)
    m_i    = alpha @ x1.T                          # [C, F]
  y = sum(fw * concat(m_i), axis=-1) + fb          # [C]
Outputs: (y [B,C], alpha_0..alpha_3 [B,C,L])

Sharding: data-parallel over batch B=32 across 8 cores (4 batches/core).
Params are replicated; embedding rows are gathered on-device via indirect
DMA.
"""

import numpy as np

import concourse.bass as bass
import concourse.bacc as bacc
import concourse.tile as tile
from concourse import mybir
from concourse.bass_utils import run_bass_kernel_spmd
from concourse.masks import make_identity

V, E, F, C, B, L = 50000, 100, 256, 50, 32, 2500
KS = (3, 5, 7, 9)
OFFS = (0, 3, 8, 15)  # tap offsets of each branch in the packed weight tensor
NTAPS = sum(KS)  # 24
NCORES = 8
NB = B // NCORES  # batches per core
PAD = 4  # max k//2
LP = L + 2 * PAD
LT = 500  # L tile for conv/logits (PSUM bank = 512 fp32)
NLT = L // LT
TB = 128  # L block for transposes / m-matmul contraction
NTB = (L + TB - 1) // TB  # 20 (last block = 68)

F32 = mybir.dt.float32
F32R = mybir.dt.float32r
BF16 = mybir.dt.bfloat16
I32 = mybir.dt.int32


def _blk(t):
    return TB if t < NTB - 1 else L - TB * (NTB - 1)


def build_kernel(nc: bass.Bass):
    d_idx = nc.dram_tensor("xidx", [NB, L], I32, kind="ExternalInput")
    d_emb = nc.dram_tensor("emb", [V, E], F32, kind="ExternalInput")
    d_wt = nc.dram_tensor("wt", [E, NTAPS, 2, 128], F32R, kind="ExternalInput")
    d_cb = nc.dram_tensor("cb", [128, 4, 2], F32, kind="ExternalInput")
    d_uw = nc.dram_tensor("uw", [128, 4, 2, C], BF16, kind="ExternalInput")
    d_fw = nc.dram_tensor("fw", [C, 4, F], F32, kind="ExternalInput")
    d_fb = nc.dram_tensor("fb", [C, 1], F32, kind="ExternalInput")
    d_att = nc.dram_tensor("att", [4, NB, C, L], F32, kind="ExternalOutput")
    d_y = nc.dram_tensor("y", [NB, C], F32, kind="ExternalOutput")

    with tile.TileContext(nc) as tc:
        with (
            tc.tile_pool(name="const", bufs=1) as const,
            tc.tile_pool(name="xc", bufs=2) as xc_pool,
            tc.tile_pool(name="idx", bufs=4) as idx_pool,
            tc.tile_pool(name="gat", bufs=4) as gat_pool,
            tc.tile_pool(name="x1", bufs=2) as x1_pool,
            tc.tile_pool(name="x1t", bufs=2) as x1t_pool,
            tc.tile_pool(name="alt", bufs=2) as alt_pool,
            tc.tile_pool(name="sm", bufs=2) as sm_pool,
            tc.tile_pool(name="small", bufs=4) as small,
            tc.tile_pool(name="cps", bufs=2, space="PSUM") as cps,
            tc.tile_pool(name="lps", bufs=2, space="PSUM") as lps,
            tc.tile_pool(name="tps", bufs=2, space="PSUM") as tps,
            tc.tile_pool(name="mps", bufs=2, space="PSUM") as mps,
        ):
            # ---- constants ----
            wt_sb = const.tile([E, NTAPS, 2, 128], F32R)
            nc.sync.dma_start(out=wt_sb, in_=d_wt[:])
            cb_sb = const.tile([128, 4, 2], F32)
            nc.sync.dma_start(out=cb_sb, in_=d_cb[:])
            uw_sb = const.tile([128, 4, 2, C], BF16)
            nc.sync.dma_start(out=uw_sb, in_=d_uw[:])
            fw_sb = const.tile([C, 4, F], F32)
            nc.sync.dma_start(out=fw_sb, in_=d_fw[:])
            fb_sb = const.tile([C, 1], F32)
            nc.sync.dma_start(out=fb_sb, in_=d_fb[:])
            id32 = const.tile([128, 128], F32)
            make_identity(nc, id32[:])
            id16 = const.tile([128, 128], BF16)
            make_identity(nc, id16[:])
            zpad = const.tile([E, PAD], F32)
            nc.vector.memset(zpad, 0.0)

            for b in range(NB):
                # ---- embedding gather -> xc [E, LP] (zero padded) ----
                xcpad = xc_pool.tile([E, LP], F32R)
                nc.vector.tensor_copy(xcpad[:, 0:PAD], zpad)
                nc.vector.tensor_copy(xcpad[:, L + PAD : LP], zpad)
                for t in range(NTB):
                    cnt = _blk(t)
                    t0 = t * TB
                    idx_sb = idx_pool.tile([TB, 1], I32, tag="idx")
                    nc.sync.dma_start(
                        out=idx_sb[:cnt], in_=d_idx[b, t0 : t0 + cnt, None]
                    )
                    emb_blk = gat_pool.tile([TB, E], F32, tag="gat")
                    nc.gpsimd.indirect_dma_start(
                        out=emb_blk[:cnt],
                        out_offset=None,
                        in_=d_emb[:],
                        in_offset=bass.IndirectOffsetOnAxis(
                            ap=idx_sb[:cnt, :1], axis=0
                        ),
                    )
                    tp = tps.tile([E, TB], F32, tag="tp")
                    nc.tensor.transpose(
                        tp[:, :cnt], emb_blk[:cnt, :], id32[:cnt, :cnt]
                    )
                    nc.vector.tensor_copy(
                        xcpad[:, PAD + t0 : PAD + t0 + cnt], tp[:, :cnt]
                    )

                yacc = small.tile([C, 4], F32, tag="yacc")

                for br in range(4):
                    k = KS[br]
                    ctr = k // 2
                    off = OFFS[br]

                    # ---- conv + tanh -> x1 [128, 2, L] bf16 ----
                    x1 = x1_pool.tile([128, 2, L], BF16, tag="x1")
                    for h in range(2):
                        for lt in range(NLT):
                            ps = cps.tile([128, LT], F32, tag="cps")
                            base = PAD + lt * LT - ctr
                            for j in range(k):
                                nc.tensor.matmul(
                                    ps,
                                    lhsT=wt_sb[:, off + j, h, :],
                                    rhs=xcpad[:, base + j : base + j + LT],
                                    start=(j == 0),
                                    stop=(j == k - 1),
                                )
                            nc.scalar.activation(
                                out=x1[:, h, lt * LT : (lt + 1) * LT],
                                in_=ps,
                                func=mybir.ActivationFunctionType.Tanh,
                                bias=cb_sb[:, br, h : h + 1],
                            )

                    # ---- x1 -> x1T [128, NTB, 256] (L on partitions) ----
                    x1T = x1t_pool.tile([128, NTB, F], BF16, tag="x1t")
                    for h in range(2):
                        for t in range(NTB):
                            cnt = _blk(t)
                            t0 = t * TB
                            tpb = tps.tile([TB, TB], BF16, tag="tp")
                            nc.tensor.transpose(
                                tpb[:cnt, :],
                                x1[:, h, t0 : t0 + cnt],
                                id16[:, :],
                            )
                            nc.vector.tensor_copy(
                                x1T[:cnt, t, h * 128 : (h + 1) * 128],
                                tpb[:cnt, :],
                            )

                    # ---- logits -> exp -> alpha ----
                    af32 = sm_pool.tile([C, L], F32, tag="af32")
                    dens = small.tile([C, NLT], F32, tag="dens")
                    for lt in range(NLT):
                        lp = lps.tile([C, LT], F32, tag="lps")
                        for h in range(2):
                            nc.tensor.matmul(
                                lp,
                                lhsT=uw_sb[:, br, h, :],
                                rhs=x1[:, h, lt * LT : (lt + 1) * LT],
                                start=(h == 0),
                                stop=(h == 1),
                            )
                        nc.scalar.activation(
                            out=af32[:, lt * LT : (lt + 1) * LT],
                            in_=lp,
                            func=mybir.ActivationFunctionType.Exp,
                            accum_out=dens[:, lt : lt + 1],
                        )
                    den = small.tile([C, 1], F32, tag="den")
                    nc.vector.reduce_sum(
                        out=den, in_=dens, axis=mybir.AxisListType.X
                    )
                    rec = small.tile([C, 1], F32, tag="rec")
                    nc.vector.reciprocal(rec, den)
                    ab16 = sm_pool.tile([C, L], BF16, tag="ab16")
                    nc.vector.tensor_scalar_mul(ab16, in0=af32, scalar1=rec)
                    nc.vector.tensor_scalar_mul(af32, in0=af32, scalar1=rec)
                    nc.sync.dma_start(out=d_att[br, b], in_=af32)

                    # ---- alpha -> alT [128, NTB, C] ----
                    alT = alt_pool.tile([TB, NTB, C], BF16, tag="alt")
                    for t in range(NTB):
                        cnt = _blk(t)
                        t0 = t * TB
                        tpa = tps.tile([TB, C, 1], BF16, tag="tp")
                        nc.tensor.transpose(
                            tpa[:cnt, :, 0],
                            ab16[:, t0 : t0 + cnt],
                            id16[:C, :C],
                        )
                        nc.vector.tensor_copy(alT[:cnt, t, :], tpa[:cnt, :, 0])

                    # ---- m = alpha @ x1.T  [C, 256] ----
                    mp = mps.tile([C, F], F32, tag="mps")
                    for t in range(NTB):
                        cnt = _blk(t)
                        nc.tensor.matmul(
                            mp,
                            lhsT=alT[:cnt, t, :],
                            rhs=x1T[:cnt, t, :],
                            start=(t == 0),
                            stop=(t == NTB - 1),
                        )

                    # ---- y partial: yacc[:, br] = sum(m * fw_br) ----
                    scr = small.tile([C, F], F32, tag="scr")
                    nc.vector.tensor_tensor(
                        out=scr,
                        in0=mp,
                        in1=fw_sb[:, br, :],
                        op=mybir.AluOpType.mult,
                    )
                    nc.vector.reduce_sum(
                        out=yacc[:, br : br + 1],
                        in_=scr,
                        axis=mybir.AxisListType.X,
                    )

                ysum = small.tile([C, 1], F32, tag="ysum")
                nc.vector.reduce_sum(
                    out=ysum, in_=yacc, axis=mybir.AxisListType.X
                )
                nc.vector.tensor_add(out=ysum, in0=ysum, in1=fb_sb[:, 0:1])
                nc.sync.dma_start(out=d_y[b, :, None], in_=ysum)

    return nc


def pack_inputs(x, embed, cws, cbs, uws, fw, fb):
    """Host-side packing of weights into device layouts. All fp32 numpy."""
    # wt[e, off+j, h, f'] = cw[h*128+f', e, j]
    wt = np.empty((E, NTAPS, 2, 128), np.float32)
    for i, cw in enumerate(cws):
        k = KS[i]
        # cw: [F, E, k] -> [E, k, F] -> [E, k, 2, 128]
        w = np.ascontiguousarray(cw.transpose(1, 2, 0)).reshape(E, k, 2, 128)
        wt[:, OFFS[i] : OFFS[i] + k] = w
    cb = np.zeros((128, 4, 2), np.float32)
    for i in range(4):
        cb[:, i, :] = cbs[i].reshape(2, 128).T
    import ml_dtypes

    uw = np.zeros((128, 4, 2, C), ml_dtypes.bfloat16)
    for i in range(4):
        # uw_i: [C, F] -> [F, C] -> [2, 128, C]
        u = np.ascontiguousarray(uws[i].T).reshape(2, 128, C)
        uw[:, i, 0, :] = u[0]
        uw[:, i, 1, :] = u[1]
    fwp = np.ascontiguousarray(fw.reshape(C, 4, F), dtype=np.float32)
    fbp = np.ascontiguousarray(fb.reshape(C, 1), dtype=np.float32)
    return wt, cb, uw, fwp, fbp


_CACHE = {}


def _get_nc():
    if "nc" not in _CACHE:
        nc = bacc.Bacc()
        build_kernel(nc)
        nc.finalize()
        _CACHE["nc"] = nc
    return _CACHE["nc"]


def make_in_maps(x, embed, cws, cbs, uws, fw, fb):
    wt, cb, uw, fwp, fbp = pack_inputs(x, embed, cws, cbs, uws, fw, fb)
    emb = np.ascontiguousarray(embed, dtype=np.float32)
    xi = np.ascontiguousarray(x, dtype=np.int32)
    in_maps = []
    for core in range(NCORES):
        in_maps.append(
            {
                "xidx": np.ascontiguousarray(xi[core * NB : (core + 1) * NB]),
                "emb": emb,
                "wt": wt,
                "cb": cb,
                "uw": uw,
                "fw": fwp,
                "fb": fbp,
            }
        )
    return in_maps


def assemble_outputs(results):
    y = np.empty((B, C), np.float32)
    atts = [np.empty((B, C, L), np.float32) for _ in range(4)]
    for core, out in enumerate(results):
        y[core * NB : (core + 1) * NB] = out["y"]
        for i in range(4):
            atts[i][core * NB : (core + 1) * NB] = out["att"][i]
    return (y, atts[0], atts[1], atts[2], atts[3])


def kernel(x, embed, cw0, cb0, cw1, cb1, cw2, cb2, cw3, cb3,
           uw0, uw1, uw2, uw3, fw, fb, _trace=False, _tmpdir=None):
    x = np.asarray(x)
    embed = np.asarray(embed, dtype=np.float32)
    cws = [np.asarray(w, dtype=np.float32) for w in (cw0, cw1, cw2, cw3)]
    cbs = [np.asarray(w, dtype=np.float32) for w in (cb0, cb1, cb2, cb3)]
    uws = [np.asarray(w, dtype=np.float32) for w in (uw0, uw1, uw2, uw3)]
    fw = np.asarray(fw, dtype=np.float32)
    fb = np.asarray(fb, dtype=np.float32)

    nc = _get_nc()
    in_maps = make_in_maps(x, embed, cws, cbs, uws, fw, fb)
    res = run_bass_kernel_spmd(
        nc,
        in_maps,
        core_ids=list(range(NCORES)),
        trace=_trace,
        tmpdir=_tmpdir,
    )
    out = assemble_outputs(res.results)
    if _trace:
        return out, res
    return out
